# revision 2
# baseline (speedup 1.0000x reference)
"""Trainium2 Bass kernel for the reservoir-computing recurrence:

    h_t = tanh(2*(h_{t-1} @ W_res + x_t))        (scan over T)
    out  = einsum('bnt,on->bot', ys, lin_w) + lin_b

Sharding: data-parallel over batch B=16 -> 2 rows per core across 8 cores.
W_res / lin_w are replicated (small).

Per-core layout: the state lives TRANSPOSED in SBUF (hist: [128 part = n
within k-chunk, 8 k-chunk blocks x (2*(U+1)) cols = (step, batch) pairs],
bf16).  Each scan step runs 64 matmuls (8 n-tiles x 8 k-chunks) with the
W tile stationary (bf16 -> fast weight load) and the 2-column state as the
moving operand, accumulating into PSUM split across 2 banks so the
x-add (DVE) + tanh (ACT) for the first half overlaps the second half's
matmuls.  The linear readout is fused: every U steps, 16 matmuls against
lin_w^T consume the freshly written hist columns.
"""

import numpy as np
import ml_dtypes

B, N, T, OUT = 16, 1024, 4096, 256
NCORES = 8
BL = B // NCORES          # 2 batch rows per core
NT = N // 128             # 8 n-tiles / k-chunks
OH = OUT // 128           # 2 output row-halves
U = 16                    # scan steps per loop iteration
NPAIR = NT * BL           # 16 (n_tile, batch) column pairs per step


def _build(t_total: int = T, u: int = U):
    import concourse.bass as bass
    import concourse.bacc as bacc
    import concourse.tile as tile
    from concourse import mybir

    f32 = mybir.dt.float32
    bf16 = mybir.dt.bfloat16

    nc = bacc.Bacc(
        "TRN2",
        target_bir_lowering=False,
        debug=False,
        enable_asserts=False,
    )

    xs_d = nc.dram_tensor("xs", [BL, N, t_total], f32, kind="ExternalInput").ap()
    w_d = nc.dram_tensor("wres", [N, N], bf16, kind="ExternalInput").ap()
    lwt_d = nc.dram_tensor("lwT", [N, OUT], bf16, kind="ExternalInput").ap()
    lb_d = nc.dram_tensor("lb", [1, OUT], f32, kind="ExternalInput").ap()
    out_d = nc.dram_tensor("outp", [BL, OUT, t_total], f32, kind="ExternalOutput").ap()

    hc = 2 * (u + 1)      # hist cols per k-chunk block: carry pair + u step pairs

    with tile.TileContext(nc) as tc:
        with (
            tc.tile_pool(name="const", bufs=1) as cpool,
            tc.tile_pool(name="state", bufs=1) as spool,
            tc.tile_pool(name="xbuf", bufs=2) as xpool,
            tc.tile_pool(name="tmp", bufs=4) as tpool,
            tc.tile_pool(name="osb", bufs=4) as opool,
            tc.tile_pool(name="ps", bufs=2, space="PSUM") as pspool,
            tc.tile_pool(name="pr", bufs=2, space="PSUM") as prpool,
        ):
            # ---- constants into SBUF ----
            w_sb = []
            lwt_sb = []
            for j in range(NT):
                wt = cpool.tile([128, N], bf16, tag=f"w{j}")
                nc.sync.dma_start(wt[:], w_d[128 * j:128 * (j + 1), :])
                w_sb.append(wt)
                lt = cpool.tile([128, OUT], bf16, tag=f"lw{j}")
                nc.sync.dma_start(lt[:], lwt_d[128 * j:128 * (j + 1), :])
                lwt_sb.append(lt)
            lb_sb = cpool.tile([128, OH], f32, tag="lb")
            for oh in range(OH):
                nc.sync.dma_start(
                    lb_sb[:, oh:oh + 1],
                    lb_d[:, 128 * oh:128 * (oh + 1)].rearrange("one p -> p one"),
                )

            # ---- persistent transposed state ----
            hist = spool.tile([128, NT * hc], bf16, tag="hist")
            hist3 = hist[:].rearrange("p (j c) -> p j c", c=hc)
            nc.vector.memzero(hist3[:, :, 0:2])  # h0 = 0 (carry cols)

            with tc.For_i(0, t_total, u, hint_engines=(mybir.EngineType.PE,)) as it:
                # x chunk for these U steps: col = pair*U + u_loc,
                # pair = 2*i + b  (matches psum column order)
                xt = xpool.tile([128, NPAIR * u], f32, tag="x")
                for i in range(NT):
                    for b in range(BL):
                        nc.sync.dma_start(
                            xt[:, (BL * i + b) * u:(BL * i + b + 1) * u],
                            xs_d[b, 128 * i:128 * (i + 1), bass.ds(it, u)],
                        )
                x3 = xt[:].rearrange("p (q s) -> p q s", s=u)

                for ul in range(u):
                    r = 2 * ul        # read col base (within block): carry or prev step
                    w = 2 * ul + 2    # write col base
                    for half in range(2):
                        ps = pspool.tile([128, 2 * NT // 2], f32, tag=f"ps{half}")
                        for il in range(NT // 2):
                            i = (NT // 2) * half + il
                            for j in range(NT):
                                nc.tensor.matmul(
                                    ps[:, 2 * il:2 * il + 2],
                                    w_sb[j][:, 128 * i:128 * (i + 1)],
                                    hist3[:, j, r:r + 2],
                                    start=(j == 0),
                                    stop=(j == NT - 1),
                                )
                        tmp = tpool.tile([128, NT], f32, tag=f"tmp{half}")
                        nc.vector.tensor_add(
                            tmp[:],
                            ps[:],
                            x3[:, (NT // 2) * BL * half:(NT // 2) * BL * (half + 1),
                               ul:ul + 1],
                        )
                        nc.scalar.activation(
                            hist3[:, (NT // 2) * half:(NT // 2) * (half + 1), w:w + 2],
                            tmp[:],
                            mybir.ActivationFunctionType.Tanh,
                            scale=2.0,
                        )

                # fused readout for these U steps
                for oh in range(OH):
                    pr = prpool.tile([128, 2 * u], f32, tag="pr")
                    for j in range(NT):
                        nc.tensor.matmul(
                            pr[:],
                            lwt_sb[j][:, 128 * oh:128 * (oh + 1)],
                            hist3[:, j, 2:hc],
                            start=(j == 0),
                            stop=(j == NT - 1),
                        )
                    osb = opool.tile([128, BL * u], f32, tag="osb")
                    pr3 = pr[:].rearrange("p (s b) -> p s b", b=BL)
                    for b in range(BL):
                        nc.scalar.activation(
                            osb[:, b * u:(b + 1) * u],
                            pr3[:, :, b:b + 1],
                            mybir.ActivationFunctionType.Identity,
                            bias=lb_sb[:, oh:oh + 1],
                        )
                    for b in range(BL):
                        nc.sync.dma_start(
                            out_d[b, 128 * oh:128 * (oh + 1), bass.ds(it, u)],
                            osb[:, b * u:(b + 1) * u],
                        )

                # carry the last state pair into the carry slot for next iter
                nc.vector.tensor_copy(hist3[:, :, 0:2], hist3[:, :, hc - 2:hc])

    nc.compile()
    return nc


_NC_CACHE = {}


def _get_nc(t_total=T, u=U):
    key = (t_total, u)
    if key not in _NC_CACHE:
        _NC_CACHE[key] = _build(t_total, u)
    return _NC_CACHE[key]


def make_in_maps(x, W_res, lin_w, lin_b, ncores=NCORES):
    wb = np.ascontiguousarray(W_res).astype(ml_dtypes.bfloat16)
    lwt = np.ascontiguousarray(lin_w.T).astype(ml_dtypes.bfloat16)
    lb = np.ascontiguousarray(lin_b.reshape(1, OUT)).astype(np.float32)
    bl = x.shape[0] // ncores
    return [
        {
            "xs": np.ascontiguousarray(x[c * bl:(c + 1) * bl]),
            "wres": wb,
            "lwT": lwt,
            "lb": lb,
        }
        for c in range(ncores)
    ]


def kernel(x, W_res, lin_w, lin_b):
    from concourse import bass_utils

    nc = _get_nc()
    in_maps = make_in_maps(x, W_res, lin_w, lin_b)
    res = bass_utils.run_bass_kernel_spmd(
        nc, in_maps, core_ids=list(range(NCORES))
    )
    return np.concatenate([res.results[c]["outp"] for c in range(NCORES)], axis=0)
